# revision 41
# baseline (speedup 1.0000x reference)
"""Trainium2 Bass kernel for the NiN-Conv2D problem.

Network: per-pixel MLP over 7x7x3 patches, independent per filter f:
  h0 = relu(P @ W0[:,:,f] + b0)   (147 -> 32)
  h1 = relu(h0 @ W1[:,:,f] + b1)  (32 -> 16)
  out = relu(h1 @ W2[:,:,f] + b2) (16 -> 1)
for B=32, H=W=32, F=128.

Strategy: data-parallel over batch across 8 NeuronCores (4 images each).
Feature-major on-chip layout (d*f on partitions, pixels on free dim).

  L0: per quad of 4 filter-groups, chunk1 = full 128x128 matmuls into two
      (128,1024) PSUM tiles; chunk2 (K rows 128..146 + bias row) 4-way
      row-tiled concurrent. Bias rides the matmul via a ones row.
  L1: 8-way PE tiling (64x32 mode): per round, 8 concurrent matmuls of
      (64 contract = 2 filters x 32 d0) x (32 out = 2 filters x 16 d1),
      row halves writing separate PSUM banks of one (128,1024) tile.
  L2: 16 block-diag matmuls, 4-way column-tiled, accumulating into one
      (128,512) out tile per pixel tile.

L1 rounds and L2 are interleaved between L0 quads so the PSUM->SBUF
evacuation engines (ScalarE+VectorE, the co-bottleneck) never starve.
ACT engine takes all biased relu ops (bias is free there); DVE takes
bias-free 1024-wide max ops. Matmul operands bf16, fp32 PSUM.
"""
import numpy as np
import ml_dtypes

import concourse.bass as bass
import concourse.mybir as mybir
from concourse import bacc, tile
from concourse import bass_utils
from concourse.bass import ts

KH, KW = 7, 7
B, H, W, C, F = 32, 32, 32, 3, 128
K, D0, D1 = 147, 32, 16
NCORES = 8
BPC = B // NCORES            # 4 images per core
NPIX = BPC * H * W           # 4096 pixels per core
PTILE = 512
NT = NPIX // PTILE           # 8 pixel tiles

BF16 = mybir.dt.bfloat16
F32 = mybir.dt.float32
NPBF16 = ml_dtypes.bfloat16


# ----------------------------------------------------------------------------
# host-side packing (layout only)
# ----------------------------------------------------------------------------

def _pack_weights(w0, b0, w1, b1, w2, b2):
    """Shared (core-independent) weight/bias packing. Returns dict of np arrays."""
    w0 = np.asarray(w0, np.float32)
    w1 = np.asarray(w1, np.float32)
    w2 = np.asarray(w2, np.float32)
    b0 = np.asarray(b0, np.float32)
    b1 = np.asarray(b1, np.float32)
    b2 = np.asarray(b2, np.float32)

    w0a = np.empty((128, 32, 128), np.float32)   # [k, group, m=fl*32+d]
    # chunk2 (K rows 128..146 + bias row) packed for 4-way row-tiled
    # concurrency: group g lives at partitions 32*(g%4)+k, cols g*128+m.
    # Row 32*(g%4)+19 carries b0 (the patch tile has ones there), so the
    # PSUM result already includes the bias and the relu op needs none.
    w0b = np.zeros((128, 32, 128), np.float32)
    for g in range(32):
        m = w0[:, :, 4 * g:4 * g + 4].transpose(0, 2, 1).reshape(K, 128)
        w0a[:, g, :] = m[:128]
        r = g % 4
        w0b[32 * r:32 * r + 19, g, :] = m[128:]
        w0b[32 * r + 19, g, :] = b0[:, 4 * g:4 * g + 4].T.reshape(128)

    # L1 weights for 8-way 64x32 PE tiling. For group g (filters 4g..4g+3):
    #   top tile   (rows 0:64)   holds filters 4g+0, 4g+1 block-diag
    #   bottom tile(rows 64:128) holds filters 4g+2, 4g+3 block-diag
    # lhsT top = w1s[0:64, 32g:32g+32], bottom = w1s[64:128, 32g:32g+32].
    w1s = np.zeros((128, 1024), np.float32)
    for g in range(32):
        for fl in range(2):
            for half in range(2):
                f = 4 * g + 2 * half + fl
                w1s[64 * half + 32 * fl:64 * half + 32 * fl + 32,
                    32 * g + 16 * fl:32 * g + 16 * fl + 16] = w1[:, :, f]

    # h1 PSUM layout per L1 round r (over quad r's groups g=4r+c):
    #   bank A (cols 0:512):   partition 32c+16fl+d1 = filter 4g+fl,   fl in 0,1
    #   bank B (cols 512:1024):partition 32c+16fl+d1 = filter 4g+2+fl
    b1s = np.empty((128, 16), np.float32)      # col 2r+half
    w2s = np.zeros((128, 512), np.float32)     # 16 blocks of 32 cols, idx 2r+half
    for r in range(8):
        for half in range(2):
            for c in range(4):
                for fl in range(2):
                    f = 16 * r + 4 * c + 2 * half + fl
                    p0 = 32 * c + 16 * fl
                    b1s[p0:p0 + 16, 2 * r + half] = b1[:, f]
                    m = 16 * (r % 2) + 4 * c + 2 * half + fl
                    w2s[p0:p0 + 16, (2 * r + half) * 32 + m] = w2[:, 0, f]
    b2s = b2.reshape(128, 1).astype(np.float32)

    return {
        "w0a": w0a.reshape(128, 4096).astype(NPBF16),
        "w0b": w0b.reshape(128, 4096).astype(NPBF16),
        "w1s": w1s.astype(NPBF16),
        "w2s": w2s.astype(NPBF16),
        "b1s": b1s, "b2s": b2s,
    }


def _im2col_T(x_core):
    """x_core (4,32,32,3) fp32 -> PT (147, 4096) with k=(kh*7+kw)*3+c."""
    xp = np.pad(np.asarray(x_core, np.float32), ((0, 0), (3, 3), (3, 3), (0, 0)))
    PT = np.empty((K, NPIX), np.float32)
    for kh in range(KH):
        for kw in range(KW):
            blk = xp[:, kh:kh + H, kw:kw + W, :]
            t = kh * 7 + kw
            PT[t * 3:t * 3 + 3] = blk.transpose(3, 0, 1, 2).reshape(3, NPIX)
    return PT


# ----------------------------------------------------------------------------
# device kernel
# ----------------------------------------------------------------------------

def _body(tc):
    nc = tc.nc
    Relu = mybir.ActivationFunctionType.Relu

    pt1 = nc.dram_tensor("pt1", [128, NPIX], BF16, kind="ExternalInput").ap()
    pt2 = nc.dram_tensor("pt2", [128, NPIX], BF16, kind="ExternalInput").ap()
    w0a = nc.dram_tensor("w0a", [128, 4096], BF16, kind="ExternalInput").ap()
    w0b = nc.dram_tensor("w0b", [128, 4096], BF16, kind="ExternalInput").ap()
    w1d = nc.dram_tensor("w1s", [128, 1024], BF16, kind="ExternalInput").ap()
    w2d = nc.dram_tensor("w2s", [128, 512], BF16, kind="ExternalInput").ap()
    b1d = nc.dram_tensor("b1s", [128, 16], F32, kind="ExternalInput").ap()
    b2d = nc.dram_tensor("b2s", [128, 1], F32, kind="ExternalInput").ap()
    out = nc.dram_tensor("out", [128, NPIX], F32, kind="ExternalOutput").ap()

    with (
        tc.tile_pool(name="consts", bufs=1) as cpool,
        tc.tile_pool(name="h0", bufs=20) as h0pool,
        tc.tile_pool(name="h1", bufs=24) as h1pool,
        tc.tile_pool(name="outs", bufs=3) as opool,
        tc.tile_pool(name="l0p", bufs=3, space="PSUM") as l0pool,
        tc.tile_pool(name="l12p", bufs=2, space="PSUM") as l1pool,
    ):
        # Input staging split across two DMA queues (sync + gpsimd) so the
        # ~4.7MB load isn't serialized on one queue; tile-0's needs go first
        # on each queue so the PE can start ~2us in.
        def load(eng, ap, shape, dt, tag):
            t = cpool.tile(shape, dt, tag=tag)
            eng.dma_start(t[:], ap)
            return t

        # queue A (sync) gets quad 0's four small pieces first so the PE can
        # start ~1us in; the bulk rides queue B (gpsimd) behind it.
        def load2(eng, dst, w, ap, tag):
            # load a [128, w] tile in two halves so the first half lands early
            t = cpool.tile([128, w], dst, tag=tag)
            eng.dma_start(t[:, 0:w // 2], ap[:, 0:w // 2])
            eng.dma_start(t[:, w // 2:w], ap[:, w // 2:w])
            return t

        # sync carries the L0 weight stream in quad-need order; gpsimd (a
        # slower queue, ~0.75us drain per transfer) carries the L1/L2
        # constants; late-needed patch tiles ride sync at the back.
        was = [None] * 4
        wbs = [None] * 4
        for i in range(4):
            was[i] = cpool.tile([128, 1024], BF16, tag=f"w0a{i}", name="w0at")
            wbs[i] = cpool.tile([128, 1024], BF16, tag=f"w0b{i}", name="w0bt")
        nc.sync.dma_start(was[0][:, 0:512], w0a[:, 0:512])      # quad 0
        nc.sync.dma_start(wbs[0][:, 0:512], w0b[:, 0:512])
        pt1s = [load(nc.sync, pt1[:, ts(0, PTILE)], [128, PTILE], BF16, "pt1_0")]
        pt2s = [load(nc.sync, pt2[:, ts(0, PTILE)], [128, PTILE], BF16, "pt2_0")]
        for q in range(1, 8):                                   # quads 1-7
            i, half = divmod(q, 2)
            nc.sync.dma_start(was[i][:, ts(half, 512)],
                              w0a[:, 1024 * i + 512 * half:1024 * i + 512 * half + 512])
            nc.sync.dma_start(wbs[i][:, ts(half, 512)],
                              w0b[:, 1024 * i + 512 * half:1024 * i + 512 * half + 512])
            if q == 2:
                pt1s.append(load(nc.sync, pt1[:, ts(1, PTILE)], [128, PTILE], BF16, "pt1_1"))
                pt2s.append(load(nc.sync, pt2[:, ts(1, PTILE)], [128, PTILE], BF16, "pt2_1"))
        for t in range(2, NT):
            pt1s.append(load(nc.sync, pt1[:, ts(t, PTILE)], [128, PTILE], BF16, f"pt1_{t}"))
            pt2s.append(load(nc.sync, pt2[:, ts(t, PTILE)], [128, PTILE], BF16, f"pt2_{t}"))
        w1s = load(nc.gpsimd, w1d, [128, 1024], BF16, "w1")
        b1s = load(nc.gpsimd, b1d, [128, 16], F32, "b1")
        w2s = load(nc.gpsimd, w2d, [128, 512], BF16, "w2")
        b2s = load(nc.gpsimd, b2d, [128, 1], F32, "b2")

        Add, Max = mybir.AluOpType.add, mybir.AluOpType.max

        def c1(t, q, r, psA, psB):
            """One chunk-1 full-array matmul (group 4q+r, K rows 0..127)."""
            g = 4 * q + r
            ps = psA if r < 2 else psB
            nc.tensor.matmul(ps[:, ts(r % 2, PTILE)],
                             was[g // 8][:, ts(g % 8, 128)],
                             pt1s[t][:], start=True, stop=False)

        def c2pack(t, q, psA, psB):
            """Chunk-2 (K rows 128..146 + bias row), 4-way row-tiled."""
            for r in range(4):
                g = 4 * q + r
                ps = psA if r < 2 else psB
                nc.tensor.matmul(ps[:, ts(r % 2, PTILE)],
                                 wbs[g // 8][32 * r:32 * r + 20, ts(g % 8, 128)],
                                 pt2s[t][32 * r:32 * r + 20, :],
                                 start=False, stop=True,
                                 tile_position=(32 * r, 0))

        def l0_act(t, q, ps, which):
            h = h0pool.tile([128, 2 * PTILE], BF16, tag="h0")
            # ScalarE gets ~4 of 16 L0 tiles/pixel-tile, DVE the rest
            # (ScalarE also carries all the biased L1/L2 relus)
            if which == 0 and q % 2 == 1:
                nc.scalar.activation(h[:], ps[:], Relu)
            else:
                nc.vector.tensor_scalar_max(h[:], ps[:], 0.0)
            return h

        def l1_pack(r, hA, hB, P, half):
            """One 4-wide tiled L1 pack: row half 0 (filters 4g,4g+1) or
            half 1 (filters 4g+2,4g+3) of quad r's groups."""
            rows = slice(64 * half, 64 * half + 64)
            for c in range(4):
                g = 4 * r + c
                src = hA if c < 2 else hB
                cols = ts(c % 2, PTILE)
                nc.tensor.matmul(P[32 * c:32 * c + 32, :],
                                 w1s[rows, ts(g, 32)], src[rows, cols],
                                 start=True, stop=True,
                                 tile_position=(64 * half, 32 * c))

        def l1_act(r, P, half, last):
            h1t = h1pool.tile([128, PTILE], BF16, tag="h1")
            bias = b1s[:, 2 * r + half:2 * r + half + 1]
            if last and half == 1:
                # drain the pipeline tail through both engines
                nc.vector.tensor_scalar(h1t[:], P[:], bias, 0.0, Add, Max)
            else:
                nc.scalar.activation(h1t[:], P[:], Relu, bias=bias)
            return h1t

        def l2_mms(O, h1, js):
            """Layer-2 matmul quarter-groups j: 4 block-diag matmuls each,
            4-way column-tiled, accumulating into O."""
            for j in js:
                for k in range(4):
                    r = 2 * k + j // 2
                    half = j % 2
                    nc.tensor.matmul(O[32 * k:32 * k + 32, :],
                                     w2s[:, ts(2 * r + half, 32)],
                                     h1[2 * r + half][:],
                                     start=(j == 0), stop=(j == 3),
                                     tile_position=(0, 32 * k))

        def l2_finish(t, O):
            ot = opool.tile([128, PTILE], F32, tag="o")
            nc.scalar.activation(ot[:], O[:], Relu, bias=b2s[:, 0:1])
            nc.sync.dma_start(out[:, ts(t, PTILE)], ot[:])

        # Quad emission: chunk2 runs FIRST (start=True) and the four
        # chunk1 matmuls accumulate (stop on their region), so psA completes
        # at c1(g1) and psB at c1(g3) — each act fires ~2 matmuls earlier
        # than with chunk2-last, freeing PSUM slots with ~1.5us of margin.
        NQ = 8 * NT
        pend = {}
        h1s = {}
        l2pend = []

        def emit_quad(t, q):
            psA = l0pool.tile([128, 2 * PTILE], F32, tag="l0", name="psA")
            psB = l0pool.tile([128, 2 * PTILE], F32, tag="l0", name="psB")
            c1(t, q, 0, psA, psB)
            c1(t, q, 1, psA, psB)
            c1(t, q, 2, psA, psB)
            c1(t, q, 3, psA, psB)
            c2pack(t, q, psA, psB)
            hA = l0_act(t, q, psA, 0)
            hB = l0_act(t, q, psB, 1)
            return hA, hB

        def issue_round(R, drain=False):
            rt, r = divmod(R, 8)
            hA, hB = pend.pop(R)
            last = R >= NQ - 3
            # the final round borrows the (by then idle) l0pool banks so it
            # never waits on the previous round's activations
            pool, tg = (l0pool, "l0") if drain else (l1pool, "l12")
            PA = pool.tile([128, PTILE], F32, tag=tg, name="PA")
            PB = pool.tile([128, PTILE], F32, tag=tg, name="PB")
            l1_pack(r, hA, hB, PA, 0)
            l1_pack(r, hA, hB, PB, 1)
            d = h1s.setdefault(rt, {})
            d[2 * r] = l1_act(r, PA, 0, last)
            d[2 * r + 1] = l1_act(r, PB, 1, last)
            if r == 7:
                l2pend.append((rt, h1s.pop(rt), R + 2))

        def flush_l2(cur_q=None):
            while l2pend and (cur_q is None or cur_q - l2pend[0][2] >= 2):
                ot, d2, _ = l2pend.pop(0)
                O = l1pool.tile([128, PTILE], F32, tag="l12", name="l2o")
                l2_mms(O, d2, (0, 1, 2, 3))
                l2_finish(ot, O)

        for t in range(NT):
            for q in range(8):
                pend[8 * t + q] = emit_quad(t, q)
                flush_l2(8 * t + q)
                if 8 * t + q >= 2:
                    issue_round(8 * t + q - 2)
        issue_round(NQ - 2)
        issue_round(NQ - 1, drain=True)
        flush_l2()


_COMPILED = None


def _get_compiled():
    global _COMPILED
    if _COMPILED is None:
        import time as _time
        t0 = _time.time()
        nc = bacc.Bacc("TRN2", target_bir_lowering=False, debug=False,
                       num_devices=NCORES)
        with tile.TileContext(nc) as tc:
            _body(tc)
        t1 = _time.time()
        nc.compile()
        t2 = _time.time()
        print(f"[kernel] tile build+schedule {t1 - t0:.1f}s, bacc compile {t2 - t1:.1f}s",
              flush=True)
        _COMPILED = nc
    return _COMPILED


# ----------------------------------------------------------------------------
# public entry point
# ----------------------------------------------------------------------------

def kernel(x, w0, b0, w1, b1, w2, b2, _trace=False):
    x = np.asarray(x, np.float32)
    shared = _pack_weights(w0, b0, w1, b1, w2, b2)

    in_maps = []
    for k in range(NCORES):
        PT = _im2col_T(x[BPC * k:BPC * (k + 1)])
        m = dict(shared)
        m["pt1"] = PT[:128].astype(NPBF16)
        # chunk2 rows replicated at partitions 32r (4-way row tiling),
        # with a ones row at 32r+19 that carries b0 through the matmul
        pt2 = np.zeros((128, NPIX), np.float32)
        for r in range(4):
            pt2[32 * r:32 * r + 19] = PT[128:]
            pt2[32 * r + 19] = 1.0
        m["pt2"] = pt2.astype(NPBF16)
        in_maps.append(m)

    import time as _time
    nc = _get_compiled()
    t0 = _time.time()
    res = bass_utils.run_bass_kernel_spmd(
        nc, in_maps, core_ids=list(range(NCORES)), trace=_trace)
    print(f"[kernel] run_bass_kernel_spmd {_time.time() - t0:.1f}s", flush=True)

    outs = []
    for k in range(NCORES):
        oc = res.results[k]["out"]                     # (128, 4096) fp32
        outs.append(oc.reshape(F, BPC, H, W).transpose(1, 2, 3, 0))
    full = np.concatenate(outs, axis=0).astype(np.float32)
    if _trace:
        return full, res
    return full


# revision 42
# speedup vs baseline: 1.1280x; 1.1280x over previous
"""Trainium2 Bass kernel for the NiN-Conv2D problem.

Network: per-pixel MLP over 7x7x3 patches, independent per filter f:
  h0 = relu(P @ W0[:,:,f] + b0)   (147 -> 32)
  h1 = relu(h0 @ W1[:,:,f] + b1)  (32 -> 16)
  out = relu(h1 @ W2[:,:,f] + b2) (16 -> 1)
for B=32, H=W=32, F=128.

Strategy: data-parallel over batch across 8 NeuronCores (4 images each).
Feature-major on-chip layout (d*f on partitions, pixels on free dim).

  L0: per quad of 4 filter-groups, chunk1 = full 128x128 matmuls into two
      (128,1024) PSUM tiles; chunk2 (K rows 128..146 + bias row) 4-way
      row-tiled concurrent. Bias rides the matmul via a ones row.
  L1: 8-way PE tiling (64x32 mode): per round, 8 concurrent matmuls of
      (64 contract = 2 filters x 32 d0) x (32 out = 2 filters x 16 d1),
      row halves writing separate PSUM banks of one (128,1024) tile.
  L2: 16 block-diag matmuls, 4-way column-tiled, accumulating into one
      (128,512) out tile per pixel tile.

L1 rounds and L2 are interleaved between L0 quads so the PSUM->SBUF
evacuation engines (ScalarE+VectorE, the co-bottleneck) never starve.
ACT engine takes all biased relu ops (bias is free there); DVE takes
bias-free 1024-wide max ops. Matmul operands bf16, fp32 PSUM.
"""
import numpy as np
import ml_dtypes

import concourse.bass as bass
import concourse.mybir as mybir
from concourse import bacc, tile
from concourse import bass_utils
from concourse.bass import ts

KH, KW = 7, 7
B, H, W, C, F = 32, 32, 32, 3, 128
K, D0, D1 = 147, 32, 16
NCORES = 8
BPC = B // NCORES            # 4 images per core
NPIX = BPC * H * W           # 4096 pixels per core
PTILE = 512
NT = NPIX // PTILE           # 8 pixel tiles

BF16 = mybir.dt.bfloat16
F32 = mybir.dt.float32
NPBF16 = ml_dtypes.bfloat16


# ----------------------------------------------------------------------------
# host-side packing (layout only)
# ----------------------------------------------------------------------------

def _pack_weights(w0, b0, w1, b1, w2, b2):
    """Shared (core-independent) weight/bias packing. Returns dict of np arrays."""
    w0 = np.asarray(w0, np.float32)
    w1 = np.asarray(w1, np.float32)
    w2 = np.asarray(w2, np.float32)
    b0 = np.asarray(b0, np.float32)
    b1 = np.asarray(b1, np.float32)
    b2 = np.asarray(b2, np.float32)

    w0a = np.empty((128, 32, 128), np.float32)   # [k, group, m=fl*32+d]
    # chunk2 (K rows 128..146 + bias row) packed for 4-way row-tiled
    # concurrency: group g lives at partitions 32*(g%4)+k, cols g*128+m.
    # Row 32*(g%4)+19 carries b0 (the patch tile has ones there), so the
    # PSUM result already includes the bias and the relu op needs none.
    w0b = np.zeros((128, 32, 128), np.float32)
    for g in range(32):
        m = w0[:, :, 4 * g:4 * g + 4].transpose(0, 2, 1).reshape(K, 128)
        w0a[:, g, :] = m[:128]
        r = g % 4
        w0b[32 * r:32 * r + 19, g, :] = m[128:]
        w0b[32 * r + 19, g, :] = b0[:, 4 * g:4 * g + 4].T.reshape(128)

    # L1 weights for 8-way 64x32 PE tiling. For group g (filters 4g..4g+3):
    #   top tile   (rows 0:64)   holds filters 4g+0, 4g+1 block-diag
    #   bottom tile(rows 64:128) holds filters 4g+2, 4g+3 block-diag
    # lhsT top = w1s[0:64, 32g:32g+32], bottom = w1s[64:128, 32g:32g+32].
    w1s = np.zeros((128, 1024), np.float32)
    for g in range(32):
        for fl in range(2):
            for half in range(2):
                f = 4 * g + 2 * half + fl
                w1s[64 * half + 32 * fl:64 * half + 32 * fl + 32,
                    32 * g + 16 * fl:32 * g + 16 * fl + 16] = w1[:, :, f]

    # h1 PSUM layout per L1 round r (over quad r's groups g=4r+c):
    #   bank A (cols 0:512):   partition 32c+16fl+d1 = filter 4g+fl,   fl in 0,1
    #   bank B (cols 512:1024):partition 32c+16fl+d1 = filter 4g+2+fl
    b1s = np.empty((128, 16), np.float32)      # col 2r+half
    w2s = np.zeros((128, 512), np.float32)     # 16 blocks of 32 cols, idx 2r+half
    for r in range(8):
        for half in range(2):
            for c in range(4):
                for fl in range(2):
                    f = 16 * r + 4 * c + 2 * half + fl
                    p0 = 32 * c + 16 * fl
                    b1s[p0:p0 + 16, 2 * r + half] = b1[:, f]
                    m = 16 * (r % 2) + 4 * c + 2 * half + fl
                    w2s[p0:p0 + 16, (2 * r + half) * 32 + m] = w2[:, 0, f]
    b2s = b2.reshape(128, 1).astype(np.float32)

    return {
        "w0a": w0a.reshape(128, 4096).astype(NPBF16),
        "w0b": w0b.reshape(128, 4096).astype(NPBF16),
        "w1s": w1s.astype(NPBF16),
        "w2s": w2s.astype(NPBF16),
        "b1s": b1s, "b2s": b2s,
    }


def _im2col_T(x_core):
    """x_core (4,32,32,3) fp32 -> PT (147, 4096) with k=(kh*7+kw)*3+c."""
    xp = np.pad(np.asarray(x_core, np.float32), ((0, 0), (3, 3), (3, 3), (0, 0)))
    PT = np.empty((K, NPIX), np.float32)
    for kh in range(KH):
        for kw in range(KW):
            blk = xp[:, kh:kh + H, kw:kw + W, :]
            t = kh * 7 + kw
            PT[t * 3:t * 3 + 3] = blk.transpose(3, 0, 1, 2).reshape(3, NPIX)
    return PT


# ----------------------------------------------------------------------------
# device kernel
# ----------------------------------------------------------------------------

def _body(tc):
    nc = tc.nc
    Relu = mybir.ActivationFunctionType.Relu

    pt1 = nc.dram_tensor("pt1", [128, NPIX], BF16, kind="ExternalInput").ap()
    pt2 = nc.dram_tensor("pt2", [128, NPIX], BF16, kind="ExternalInput").ap()
    w0a = nc.dram_tensor("w0a", [128, 4096], BF16, kind="ExternalInput").ap()
    w0b = nc.dram_tensor("w0b", [128, 4096], BF16, kind="ExternalInput").ap()
    w1d = nc.dram_tensor("w1s", [128, 1024], BF16, kind="ExternalInput").ap()
    w2d = nc.dram_tensor("w2s", [128, 512], BF16, kind="ExternalInput").ap()
    b1d = nc.dram_tensor("b1s", [128, 16], F32, kind="ExternalInput").ap()
    b2d = nc.dram_tensor("b2s", [128, 1], F32, kind="ExternalInput").ap()
    out = nc.dram_tensor("out", [128, NPIX], F32, kind="ExternalOutput").ap()

    with (
        tc.tile_pool(name="consts", bufs=1) as cpool,
        tc.tile_pool(name="h0", bufs=20) as h0pool,
        tc.tile_pool(name="h1", bufs=24) as h1pool,
        tc.tile_pool(name="outs", bufs=3) as opool,
        tc.tile_pool(name="l0p", bufs=3, space="PSUM") as l0pool,
        tc.tile_pool(name="l12p", bufs=2, space="PSUM") as l1pool,
    ):
        # Input staging split across two DMA queues (sync + gpsimd) so the
        # ~4.7MB load isn't serialized on one queue; tile-0's needs go first
        # on each queue so the PE can start ~2us in.
        def load(eng, ap, shape, dt, tag):
            t = cpool.tile(shape, dt, tag=tag)
            eng.dma_start(t[:], ap)
            return t

        # queue A (sync) gets quad 0's four small pieces first so the PE can
        # start ~1us in; the bulk rides queue B (gpsimd) behind it.
        def load2(eng, dst, w, ap, tag):
            # load a [128, w] tile in two halves so the first half lands early
            t = cpool.tile([128, w], dst, tag=tag)
            eng.dma_start(t[:, 0:w // 2], ap[:, 0:w // 2])
            eng.dma_start(t[:, w // 2:w], ap[:, w // 2:w])
            return t

        # sync carries the L0 weight stream in quad-need order; gpsimd (a
        # slower queue, ~0.75us drain per transfer) carries the L1/L2
        # constants; late-needed patch tiles ride sync at the back.
        was = [None] * 4
        wbs = [None] * 4
        for i in range(4):
            was[i] = cpool.tile([128, 1024], BF16, tag=f"w0a{i}", name="w0at")
            wbs[i] = cpool.tile([128, 1024], BF16, tag=f"w0b{i}", name="w0bt")
        nc.sync.dma_start(was[0][:, 0:512], w0a[:, 0:512])      # quad 0
        nc.sync.dma_start(wbs[0][:, 0:512], w0b[:, 0:512])
        pt1s = [load(nc.sync, pt1[:, ts(0, PTILE)], [128, PTILE], BF16, "pt1_0")]
        pt2s = [load(nc.sync, pt2[:, ts(0, PTILE)], [128, PTILE], BF16, "pt2_0")]
        for q in range(1, 8):                                   # quads 1-7
            i, half = divmod(q, 2)
            nc.sync.dma_start(was[i][:, ts(half, 512)],
                              w0a[:, 1024 * i + 512 * half:1024 * i + 512 * half + 512])
            nc.sync.dma_start(wbs[i][:, ts(half, 512)],
                              w0b[:, 1024 * i + 512 * half:1024 * i + 512 * half + 512])
            if q == 2:
                pt1s.append(load(nc.sync, pt1[:, ts(1, PTILE)], [128, PTILE], BF16, "pt1_1"))
                pt2s.append(load(nc.sync, pt2[:, ts(1, PTILE)], [128, PTILE], BF16, "pt2_1"))
        for t in range(2, NT):
            pt1s.append(load(nc.sync, pt1[:, ts(t, PTILE)], [128, PTILE], BF16, f"pt1_{t}"))
            pt2s.append(load(nc.sync, pt2[:, ts(t, PTILE)], [128, PTILE], BF16, f"pt2_{t}"))
        w1s = load(nc.gpsimd, w1d, [128, 1024], BF16, "w1")
        b1s = load(nc.gpsimd, b1d, [128, 16], F32, "b1")
        w2s = load(nc.gpsimd, w2d, [128, 512], BF16, "w2")
        b2s = load(nc.gpsimd, b2d, [128, 1], F32, "b2")

        Add, Max = mybir.AluOpType.add, mybir.AluOpType.max

        def c1(t, q, r, psA, psB):
            """One chunk-1 full-array matmul (group 4q+r, K rows 0..127)."""
            g = 4 * q + r
            ps = psA if r < 2 else psB
            nc.tensor.matmul(ps[:, ts(r % 2, PTILE)],
                             was[g // 8][:, ts(g % 8, 128)],
                             pt1s[t][:], start=True, stop=False)

        def c2pack(t, q, psA, psB):
            """Chunk-2 (K rows 128..146 + bias row), 4-way row-tiled."""
            for r in range(4):
                g = 4 * q + r
                ps = psA if r < 2 else psB
                nc.tensor.matmul(ps[:, ts(r % 2, PTILE)],
                                 wbs[g // 8][32 * r:32 * r + 20, ts(g % 8, 128)],
                                 pt2s[t][32 * r:32 * r + 20, :],
                                 start=False, stop=True,
                                 tile_position=(32 * r, 0))

        def l0_act(t, q, ps, which):
            h = h0pool.tile([128, 2 * PTILE], BF16, tag="h0")
            # ScalarE gets ~4 of 16 L0 tiles/pixel-tile, DVE the rest
            # (ScalarE also carries all the biased L1/L2 relus)
            if which == 0 and q % 2 == 1:
                nc.scalar.activation(h[:], ps[:], Relu)
            else:
                nc.vector.tensor_scalar_max(h[:], ps[:], 0.0)
            return h

        def l1_pack(r, hA, hB, P, half):
            """One 4-wide tiled L1 pack: row half 0 (filters 4g,4g+1) or
            half 1 (filters 4g+2,4g+3) of quad r's groups."""
            rows = slice(64 * half, 64 * half + 64)
            for c in range(4):
                g = 4 * r + c
                src = hA if c < 2 else hB
                cols = ts(c % 2, PTILE)
                nc.tensor.matmul(P[32 * c:32 * c + 32, :],
                                 w1s[rows, ts(g, 32)], src[rows, cols],
                                 start=True, stop=True,
                                 tile_position=(64 * half, 32 * c))

        def l1_act(r, P, half, last):
            h1t = h1pool.tile([128, PTILE], BF16, tag="h1")
            bias = b1s[:, 2 * r + half:2 * r + half + 1]
            if last and half == 1:
                # drain the pipeline tail through both engines
                nc.vector.tensor_scalar(h1t[:], P[:], bias, 0.0, Add, Max)
            else:
                nc.scalar.activation(h1t[:], P[:], Relu, bias=bias)
            return h1t

        def l2_mms(O, h1, js):
            """Layer-2 matmul quarter-groups j: 4 block-diag matmuls each,
            4-way column-tiled, accumulating into O."""
            for j in js:
                for k in range(4):
                    r = 2 * k + j // 2
                    half = j % 2
                    nc.tensor.matmul(O[32 * k:32 * k + 32, :],
                                     w2s[:, ts(2 * r + half, 32)],
                                     h1[2 * r + half][:],
                                     start=(j == 0), stop=(j == 3),
                                     tile_position=(0, 32 * k))

        def l2_finish(t, O):
            ot = opool.tile([128, PTILE], F32, tag="o")
            nc.scalar.activation(ot[:], O[:], Relu, bias=b2s[:, 0:1])
            nc.sync.dma_start(out[:, ts(t, PTILE)], ot[:])

        # Quad emission: chunk2 runs FIRST (start=True) and the four
        # chunk1 matmuls accumulate (stop on their region), so psA completes
        # at c1(g1) and psB at c1(g3) — each act fires ~2 matmuls earlier
        # than with chunk2-last, freeing PSUM slots with ~1.5us of margin.
        NQ = 8 * NT
        pend = {}
        h1s = {}
        l2pend = []

        def emit_quad(t, q):
            psA = l0pool.tile([128, 2 * PTILE], F32, tag="l0", name="psA")
            psB = l0pool.tile([128, 2 * PTILE], F32, tag="l0", name="psB")
            c1(t, q, 0, psA, psB)
            c1(t, q, 1, psA, psB)
            c1(t, q, 2, psA, psB)
            c1(t, q, 3, psA, psB)
            c2pack(t, q, psA, psB)
            hA = l0_act(t, q, psA, 0)
            hB = l0_act(t, q, psB, 1)
            return hA, hB

        def issue_round(R, drain=False):
            rt, r = divmod(R, 8)
            hA, hB = pend.pop(R)
            last = R >= NQ - 3
            # the final round borrows the (by then idle) l0pool banks so it
            # never waits on the previous round's activations
            pool, tg = (l0pool, "l0") if drain else (l1pool, "l12")
            PA = pool.tile([128, PTILE], F32, tag=tg, name="PA")
            PB = pool.tile([128, PTILE], F32, tag=tg, name="PB")
            l1_pack(r, hA, hB, PA, 0)
            l1_pack(r, hA, hB, PB, 1)
            d = h1s.setdefault(rt, {})
            d[2 * r] = l1_act(r, PA, 0, last)
            d[2 * r + 1] = l1_act(r, PB, 1, last)
            if r == 7:
                l2pend.append((rt, h1s.pop(rt), R + 2))

        def flush_l2(cur_q=None):
            while l2pend and (cur_q is None or cur_q - l2pend[0][2] >= 2):
                ot, d2, _ = l2pend.pop(0)
                O = l1pool.tile([128, PTILE], F32, tag="l12", name="l2o")
                l2_mms(O, d2, (0, 1, 2, 3))
                l2_finish(ot, O)

        for t in range(NT):
            for q in range(8):
                pend[8 * t + q] = emit_quad(t, q)
                if 8 * t + q >= 2:
                    issue_round(8 * t + q - 2)
                flush_l2(8 * t + q)
        issue_round(NQ - 2)
        issue_round(NQ - 1, drain=True)
        flush_l2()


_COMPILED = None


def _get_compiled():
    global _COMPILED
    if _COMPILED is None:
        import time as _time
        t0 = _time.time()
        nc = bacc.Bacc("TRN2", target_bir_lowering=False, debug=False,
                       num_devices=NCORES)
        with tile.TileContext(nc) as tc:
            _body(tc)
        t1 = _time.time()
        nc.compile()
        t2 = _time.time()
        print(f"[kernel] tile build+schedule {t1 - t0:.1f}s, bacc compile {t2 - t1:.1f}s",
              flush=True)
        _COMPILED = nc
    return _COMPILED


# ----------------------------------------------------------------------------
# public entry point
# ----------------------------------------------------------------------------

def kernel(x, w0, b0, w1, b1, w2, b2, _trace=False):
    x = np.asarray(x, np.float32)
    shared = _pack_weights(w0, b0, w1, b1, w2, b2)

    in_maps = []
    for k in range(NCORES):
        PT = _im2col_T(x[BPC * k:BPC * (k + 1)])
        m = dict(shared)
        m["pt1"] = PT[:128].astype(NPBF16)
        # chunk2 rows replicated at partitions 32r (4-way row tiling),
        # with a ones row at 32r+19 that carries b0 through the matmul
        pt2 = np.zeros((128, NPIX), np.float32)
        for r in range(4):
            pt2[32 * r:32 * r + 19] = PT[128:]
            pt2[32 * r + 19] = 1.0
        m["pt2"] = pt2.astype(NPBF16)
        in_maps.append(m)

    import time as _time
    nc = _get_compiled()
    t0 = _time.time()
    res = bass_utils.run_bass_kernel_spmd(
        nc, in_maps, core_ids=list(range(NCORES)), trace=_trace)
    print(f"[kernel] run_bass_kernel_spmd {_time.time() - t0:.1f}s", flush=True)

    outs = []
    for k in range(NCORES):
        oc = res.results[k]["out"]                     # (128, 4096) fp32
        outs.append(oc.reshape(F, BPC, H, W).transpose(1, 2, 3, 0))
    full = np.concatenate(outs, axis=0).astype(np.float32)
    if _trace:
        return full, res
    return full


# revision 44
# speedup vs baseline: 1.2163x; 1.0783x over previous
"""Trainium2 Bass kernel for the NiN-Conv2D problem.

Network: per-pixel MLP over 7x7x3 patches, independent per filter f:
  h0 = relu(P @ W0[:,:,f] + b0)   (147 -> 32)
  h1 = relu(h0 @ W1[:,:,f] + b1)  (32 -> 16)
  out = relu(h1 @ W2[:,:,f] + b2) (16 -> 1)
for B=32, H=W=32, F=128.

Strategy: data-parallel over batch across 8 NeuronCores (4 images each).
Feature-major on-chip layout (d*f on partitions, pixels on free dim).

  L0: per quad of 4 filter-groups, chunk1 = full 128x128 matmuls into two
      (128,1024) PSUM tiles; chunk2 (K rows 128..146 + bias row) 4-way
      row-tiled concurrent. Bias rides the matmul via a ones row.
  L1: 8-way PE tiling (64x32 mode): per round, 8 concurrent matmuls of
      (64 contract = 2 filters x 32 d0) x (32 out = 2 filters x 16 d1),
      row halves writing separate PSUM banks of one (128,1024) tile.
  L2: 16 block-diag matmuls, 4-way column-tiled, accumulating into one
      (128,512) out tile per pixel tile.

L1 rounds and L2 are interleaved between L0 quads so the PSUM->SBUF
evacuation engines (ScalarE+VectorE, the co-bottleneck) never starve.
ACT engine takes all biased relu ops (bias is free there); DVE takes
bias-free 1024-wide max ops. Matmul operands bf16, fp32 PSUM.
"""
import numpy as np
import ml_dtypes

import concourse.bass as bass
import concourse.mybir as mybir
from concourse import bacc, tile
from concourse import bass_utils
from concourse.bass import ts

KH, KW = 7, 7
B, H, W, C, F = 32, 32, 32, 3, 128
K, D0, D1 = 147, 32, 16
NCORES = 8
BPC = B // NCORES            # 4 images per core
NPIX = BPC * H * W           # 4096 pixels per core
PTILE = 512
NT = NPIX // PTILE           # 8 pixel tiles

BF16 = mybir.dt.bfloat16
F32 = mybir.dt.float32
NPBF16 = ml_dtypes.bfloat16


# ----------------------------------------------------------------------------
# host-side packing (layout only)
# ----------------------------------------------------------------------------

def _pack_weights(w0, b0, w1, b1, w2, b2):
    """Shared (core-independent) weight/bias packing. Returns dict of np arrays."""
    w0 = np.asarray(w0, np.float32)
    w1 = np.asarray(w1, np.float32)
    w2 = np.asarray(w2, np.float32)
    b0 = np.asarray(b0, np.float32)
    b1 = np.asarray(b1, np.float32)
    b2 = np.asarray(b2, np.float32)

    w0a = np.empty((128, 32, 128), np.float32)   # [k, group, m=fl*32+d]
    # chunk2 (K rows 128..146 + bias row) packed for 4-way row-tiled
    # concurrency: group g lives at partitions 32*(g%4)+k, cols g*128+m.
    # Row 32*(g%4)+19 carries b0 (the patch tile has ones there), so the
    # PSUM result already includes the bias and the relu op needs none.
    w0b = np.zeros((128, 32, 128), np.float32)
    for g in range(32):
        m = w0[:, :, 4 * g:4 * g + 4].transpose(0, 2, 1).reshape(K, 128)
        w0a[:, g, :] = m[:128]
        r = g % 4
        w0b[32 * r:32 * r + 19, g, :] = m[128:]
        w0b[32 * r + 19, g, :] = b0[:, 4 * g:4 * g + 4].T.reshape(128)

    # L1 weights for 8-way 64x32 PE tiling. For group g (filters 4g..4g+3):
    #   top tile   (rows 0:64)   holds filters 4g+0, 4g+1 block-diag
    #   bottom tile(rows 64:128) holds filters 4g+2, 4g+3 block-diag
    # lhsT top = w1s[0:64, 32g:32g+32], bottom = w1s[64:128, 32g:32g+32].
    w1s = np.zeros((128, 1024), np.float32)
    for g in range(32):
        for fl in range(2):
            for half in range(2):
                f = 4 * g + 2 * half + fl
                w1s[64 * half + 32 * fl:64 * half + 32 * fl + 32,
                    32 * g + 16 * fl:32 * g + 16 * fl + 16] = w1[:, :, f]

    # h1 PSUM layout per L1 round r (over quad r's groups g=4r+c):
    #   bank A (cols 0:512):   partition 32c+16fl+d1 = filter 4g+fl,   fl in 0,1
    #   bank B (cols 512:1024):partition 32c+16fl+d1 = filter 4g+2+fl
    b1s = np.empty((128, 16), np.float32)      # col 2r+half
    w2s = np.zeros((128, 512), np.float32)     # 16 blocks of 32 cols, idx 2r+half
    for r in range(8):
        for half in range(2):
            for c in range(4):
                for fl in range(2):
                    f = 16 * r + 4 * c + 2 * half + fl
                    p0 = 32 * c + 16 * fl
                    b1s[p0:p0 + 16, 2 * r + half] = b1[:, f]
                    m = 16 * (r % 2) + 4 * c + 2 * half + fl
                    w2s[p0:p0 + 16, (2 * r + half) * 32 + m] = w2[:, 0, f]
    b2s = b2.reshape(128, 1).astype(np.float32)

    return {
        "w0a": w0a.reshape(128, 4096).astype(NPBF16),
        "w0b": w0b.reshape(128, 4096).astype(NPBF16),
        "w1s": w1s.astype(NPBF16),
        "w2s": w2s.astype(NPBF16),
        "b1s": b1s, "b2s": b2s,
    }


def _im2col_T(x_core):
    """x_core (4,32,32,3) fp32 -> PT (147, 4096) with k=(kh*7+kw)*3+c."""
    xp = np.pad(np.asarray(x_core, np.float32), ((0, 0), (3, 3), (3, 3), (0, 0)))
    PT = np.empty((K, NPIX), np.float32)
    for kh in range(KH):
        for kw in range(KW):
            blk = xp[:, kh:kh + H, kw:kw + W, :]
            t = kh * 7 + kw
            PT[t * 3:t * 3 + 3] = blk.transpose(3, 0, 1, 2).reshape(3, NPIX)
    return PT


# ----------------------------------------------------------------------------
# device kernel
# ----------------------------------------------------------------------------

def _body(tc):
    nc = tc.nc
    Relu = mybir.ActivationFunctionType.Relu

    pt1 = nc.dram_tensor("pt1", [128, NPIX], BF16, kind="ExternalInput").ap()
    pt2 = nc.dram_tensor("pt2", [128, NPIX], BF16, kind="ExternalInput").ap()
    w0a = nc.dram_tensor("w0a", [128, 4096], BF16, kind="ExternalInput").ap()
    w0b = nc.dram_tensor("w0b", [128, 4096], BF16, kind="ExternalInput").ap()
    w1d = nc.dram_tensor("w1s", [128, 1024], BF16, kind="ExternalInput").ap()
    w2d = nc.dram_tensor("w2s", [128, 512], BF16, kind="ExternalInput").ap()
    b1d = nc.dram_tensor("b1s", [128, 16], F32, kind="ExternalInput").ap()
    b2d = nc.dram_tensor("b2s", [128, 1], F32, kind="ExternalInput").ap()
    out = nc.dram_tensor("out", [128, NPIX], F32, kind="ExternalOutput").ap()

    with (
        tc.tile_pool(name="consts", bufs=1) as cpool,
        tc.tile_pool(name="h0", bufs=20) as h0pool,
        tc.tile_pool(name="h1", bufs=28) as h1pool,
        tc.tile_pool(name="outs", bufs=4) as opool,
        tc.tile_pool(name="l0p", bufs=3, space="PSUM") as l0pool,
        tc.tile_pool(name="l12p", bufs=2, space="PSUM") as l1pool,
    ):
        # Input staging split across two DMA queues (sync + gpsimd) so the
        # ~4.7MB load isn't serialized on one queue; tile-0's needs go first
        # on each queue so the PE can start ~2us in.
        def load(eng, ap, shape, dt, tag):
            t = cpool.tile(shape, dt, tag=tag)
            eng.dma_start(t[:], ap)
            return t

        # queue A (sync) gets quad 0's four small pieces first so the PE can
        # start ~1us in; the bulk rides queue B (gpsimd) behind it.
        def load2(eng, dst, w, ap, tag):
            # load a [128, w] tile in two halves so the first half lands early
            t = cpool.tile([128, w], dst, tag=tag)
            eng.dma_start(t[:, 0:w // 2], ap[:, 0:w // 2])
            eng.dma_start(t[:, w // 2:w], ap[:, w // 2:w])
            return t

        # sync carries the L0 weight stream in quad-need order; gpsimd (a
        # slower queue, ~0.75us drain per transfer) carries the L1/L2
        # constants; late-needed patch tiles ride sync at the back.
        was = [None] * 4
        wbs = [None] * 4
        for i in range(4):
            was[i] = cpool.tile([128, 1024], BF16, tag=f"w0a{i}", name="w0at")
            wbs[i] = cpool.tile([128, 1024], BF16, tag=f"w0b{i}", name="w0bt")
        nc.sync.dma_start(was[0][:, 0:512], w0a[:, 0:512])      # quad 0
        nc.sync.dma_start(wbs[0][:, 0:512], w0b[:, 0:512])
        pt1s = [load(nc.sync, pt1[:, ts(0, PTILE)], [128, PTILE], BF16, "pt1_0")]
        pt2s = [load(nc.sync, pt2[:, ts(0, PTILE)], [128, PTILE], BF16, "pt2_0")]
        for q in range(1, 8):                                   # quads 1-7
            i, half = divmod(q, 2)
            nc.sync.dma_start(was[i][:, ts(half, 512)],
                              w0a[:, 1024 * i + 512 * half:1024 * i + 512 * half + 512])
            nc.sync.dma_start(wbs[i][:, ts(half, 512)],
                              w0b[:, 1024 * i + 512 * half:1024 * i + 512 * half + 512])
            if q == 2:
                pt1s.append(load(nc.sync, pt1[:, ts(1, PTILE)], [128, PTILE], BF16, "pt1_1"))
                pt2s.append(load(nc.sync, pt2[:, ts(1, PTILE)], [128, PTILE], BF16, "pt2_1"))
        for t in range(2, NT):
            pt1s.append(load(nc.sync, pt1[:, ts(t, PTILE)], [128, PTILE], BF16, f"pt1_{t}"))
            pt2s.append(load(nc.sync, pt2[:, ts(t, PTILE)], [128, PTILE], BF16, f"pt2_{t}"))
        w1s = load(nc.gpsimd, w1d, [128, 1024], BF16, "w1")
        b1s = load(nc.gpsimd, b1d, [128, 16], F32, "b1")
        w2s = load(nc.gpsimd, w2d, [128, 512], BF16, "w2")
        b2s = load(nc.gpsimd, b2d, [128, 1], F32, "b2")

        Add, Max = mybir.AluOpType.add, mybir.AluOpType.max

        def c1(t, q, r, psA, psB):
            """One chunk-1 full-array matmul (group 4q+r, K rows 0..127)."""
            g = 4 * q + r
            ps = psA if r < 2 else psB
            nc.tensor.matmul(ps[:, ts(r % 2, PTILE)],
                             was[g // 8][:, ts(g % 8, 128)],
                             pt1s[t][:], start=True, stop=False)

        def c2pack(t, q, psA, psB):
            """Chunk-2 (K rows 128..146 + bias row), 4-way row-tiled."""
            for r in range(4):
                g = 4 * q + r
                ps = psA if r < 2 else psB
                nc.tensor.matmul(ps[:, ts(r % 2, PTILE)],
                                 wbs[g // 8][32 * r:32 * r + 20, ts(g % 8, 128)],
                                 pt2s[t][32 * r:32 * r + 20, :],
                                 start=False, stop=True,
                                 tile_position=(32 * r, 0))

        def l0_act(t, q, ps, which):
            h = h0pool.tile([128, 2 * PTILE], BF16, tag="h0")
            # ScalarE gets ~4 of 16 L0 tiles/pixel-tile, DVE the rest
            # (ScalarE also carries all the biased L1/L2 relus)
            if which == 0 and q % 2 == 1:
                nc.scalar.activation(h[:], ps[:], Relu)
            else:
                nc.vector.tensor_scalar_max(h[:], ps[:], 0.0)
            return h

        def l1_pack(r, hA, hB, P, half):
            """One 4-wide tiled L1 pack: row half 0 (filters 4g,4g+1) or
            half 1 (filters 4g+2,4g+3) of quad r's groups."""
            rows = slice(64 * half, 64 * half + 64)
            for c in range(4):
                g = 4 * r + c
                src = hA if c < 2 else hB
                cols = ts(c % 2, PTILE)
                nc.tensor.matmul(P[32 * c:32 * c + 32, :],
                                 w1s[rows, ts(g, 32)], src[rows, cols],
                                 start=True, stop=True,
                                 tile_position=(64 * half, 32 * c))

        def l1_act(r, P, half, last):
            h1t = h1pool.tile([128, PTILE], BF16, tag="h1")
            bias = b1s[:, 2 * r + half:2 * r + half + 1]
            if last and half == 1:
                # drain the pipeline tail through both engines
                nc.vector.tensor_scalar(h1t[:], P[:], bias, 0.0, Add, Max)
            else:
                nc.scalar.activation(h1t[:], P[:], Relu, bias=bias)
            return h1t

        def l2_mms(O, h1, js):
            """Layer-2 matmul quarter-groups j: 4 block-diag matmuls each,
            4-way column-tiled, accumulating into O."""
            for j in js:
                for k in range(4):
                    r = 2 * k + j // 2
                    half = j % 2
                    nc.tensor.matmul(O[32 * k:32 * k + 32, :],
                                     w2s[:, ts(2 * r + half, 32)],
                                     h1[2 * r + half][:],
                                     start=(j == 0), stop=(j == 3),
                                     tile_position=(0, 32 * k))

        def l2_finish(t, O):
            ot = opool.tile([128, PTILE], F32, tag="o")
            nc.scalar.activation(ot[:], O[:], Relu, bias=b2s[:, 0:1])
            nc.sync.dma_start(out[:, ts(t, PTILE)], ot[:])

        # Quad emission: chunk2 runs FIRST (start=True) and the four
        # chunk1 matmuls accumulate (stop on their region), so psA completes
        # at c1(g1) and psB at c1(g3) — each act fires ~2 matmuls earlier
        # than with chunk2-last, freeing PSUM slots with ~1.5us of margin.
        NQ = 8 * NT
        pend = {}
        h1s = {}
        l2pend = []

        def emit_quad(t, q):
            psA = l0pool.tile([128, 2 * PTILE], F32, tag="l0", name="psA")
            psB = l0pool.tile([128, 2 * PTILE], F32, tag="l0", name="psB")
            c1(t, q, 0, psA, psB)
            c1(t, q, 1, psA, psB)
            c1(t, q, 2, psA, psB)
            c1(t, q, 3, psA, psB)
            c2pack(t, q, psA, psB)
            hA = l0_act(t, q, psA, 0)
            hB = l0_act(t, q, psB, 1)
            return hA, hB

        def issue_round(R, drain=False):
            rt, r = divmod(R, 8)
            hA, hB = pend.pop(R)
            last = R >= NQ - 3
            # the final round borrows the (by then idle) l0pool banks so it
            # never waits on the previous round's activations
            pool, tg = (l0pool, "l0") if drain else (l1pool, "l12")
            PA = pool.tile([128, PTILE], F32, tag=tg, name="PA")
            PB = pool.tile([128, PTILE], F32, tag=tg, name="PB")
            l1_pack(r, hA, hB, PA, 0)
            l1_pack(r, hA, hB, PB, 1)
            d = h1s.setdefault(rt, {})
            d[2 * r] = l1_act(r, PA, 0, last)
            d[2 * r + 1] = l1_act(r, PB, 1, last)
            if r == 7:
                l2pend.append((rt, h1s.pop(rt), R + 2))

        def flush_l2(cur_q=None):
            while l2pend and (cur_q is None or cur_q - l2pend[0][2] >= 2):
                ot, d2, _ = l2pend.pop(0)
                O = l1pool.tile([128, PTILE], F32, tag="l12", name="l2o")
                l2_mms(O, d2, (0, 1, 2, 3))
                l2_finish(ot, O)

        for t in range(NT):
            for q in range(8):
                pend[8 * t + q] = emit_quad(t, q)
                flush_l2(8 * t + q)
                if 8 * t + q >= 2:
                    issue_round(8 * t + q - 2)
        issue_round(NQ - 2)
        issue_round(NQ - 1, drain=True)
        flush_l2()


_COMPILED = None


def _get_compiled():
    global _COMPILED
    if _COMPILED is None:
        import time as _time
        t0 = _time.time()
        nc = bacc.Bacc("TRN2", target_bir_lowering=False, debug=False,
                       num_devices=NCORES)
        with tile.TileContext(nc) as tc:
            _body(tc)
        t1 = _time.time()
        nc.compile()
        t2 = _time.time()
        print(f"[kernel] tile build+schedule {t1 - t0:.1f}s, bacc compile {t2 - t1:.1f}s",
              flush=True)
        _COMPILED = nc
    return _COMPILED


# ----------------------------------------------------------------------------
# public entry point
# ----------------------------------------------------------------------------

def kernel(x, w0, b0, w1, b1, w2, b2, _trace=False):
    x = np.asarray(x, np.float32)
    shared = _pack_weights(w0, b0, w1, b1, w2, b2)

    in_maps = []
    for k in range(NCORES):
        PT = _im2col_T(x[BPC * k:BPC * (k + 1)])
        m = dict(shared)
        m["pt1"] = PT[:128].astype(NPBF16)
        # chunk2 rows replicated at partitions 32r (4-way row tiling),
        # with a ones row at 32r+19 that carries b0 through the matmul
        pt2 = np.zeros((128, NPIX), np.float32)
        for r in range(4):
            pt2[32 * r:32 * r + 19] = PT[128:]
            pt2[32 * r + 19] = 1.0
        m["pt2"] = pt2.astype(NPBF16)
        in_maps.append(m)

    import time as _time
    nc = _get_compiled()
    t0 = _time.time()
    res = bass_utils.run_bass_kernel_spmd(
        nc, in_maps, core_ids=list(range(NCORES)), trace=_trace)
    print(f"[kernel] run_bass_kernel_spmd {_time.time() - t0:.1f}s", flush=True)

    outs = []
    for k in range(NCORES):
        oc = res.results[k]["out"]                     # (128, 4096) fp32
        outs.append(oc.reshape(F, BPC, H, W).transpose(1, 2, 3, 0))
    full = np.concatenate(outs, axis=0).astype(np.float32)
    if _trace:
        return full, res
    return full
